# revision 30
# baseline (speedup 1.0000x reference)
"""DistanceFromAnswerLoss on 8 Trainium2 NeuronCores.

out = 0.1 * sum_{b,c} mask[b,c] * exp(input[b,c])
  mask[b,c] = |c - t_b| / sqrt(sum_c (c - t_b)^2),  mask = 0 where t_b == 0

Sharding: data-parallel over the batch dim (512 rows per core); each core
computes its partial 0.1 * sum, host adds the 8 scalars.

Per-core pipeline (memory-bound target: ~47us DMA floor at 360 GB/s):
  ScalarE : e = exp(x) -> bf16 (one pass, 1.2 GHz, table set exp_and_others)
  VectorE : d = iota - t        (tensor_scalar ptr, 2x/4x packed bf16 mode)
            p = d * e           (tensor_tensor, 2x bf16 mode)
  abs+row-reduce, split to balance engines:
    K_ACT tiles: ScalarE activation(Abs, accum_out) — same table set as exp,
                 so no table reloads; gives sum|p| per partition for free.
    rest:        two scalar_tensor_tensor ops (1x) using
                 |c-t|*e = max(c,t)*e - min(c,t)*e.
The row norm uses the closed form sum_c (c-t)^2 = C*(t-mu)^2 + K computed
once for all 512 rows on [128,4] tiles.  Final 128-partition reduction is a
tiny matmul against a ones vector on the otherwise-idle PE.
"""

import sys
from contextlib import ExitStack

import numpy as np

sys.path.insert(0, "/opt/trn_rl_repo")

import concourse.bass as bass
import concourse.tile as tile
from concourse import bacc, mybir
from concourse.bass_utils import run_bass_kernel_spmd

B = 4096
C = 8192
N_CORES = 8
ROWS = B // N_CORES          # 512 rows per core
RB = ROWS // 128             # 4 row blocks of 128 partitions
W = 8192                     # column tile width (4 MiB DMAs)
NW = C // W
NT = RB * NW                 # 8 big tiles per core
ACT_IDXS = {0, 2}            # tiles whose abs+reduce runs on ScalarE
# others: tensor_reduce(apply_absolute_value) on VectorE
COEFF = 0.1

MU = (C - 1) / 2.0
_S1 = (C - 1) * C // 2
_S2 = (C - 1) * C * (2 * C - 1) // 6
K = float(_S2 - _S1 * _S1 / C)   # sum_c (c-t)^2 = C*(t-MU)^2 + K

F32 = mybir.dt.float32
BF16 = mybir.dt.bfloat16
Af = mybir.ActivationFunctionType
Op = mybir.AluOpType


def _build() -> bass.Bass:
    nc = bacc.Bacc("TRN2", target_bir_lowering=False, debug=False)
    x = nc.declare_dram_parameter("x", [RB, 128, C], F32, isOutput=False)
    # t arrives twice: partition-major [128, RB] for the batched norm chain,
    # and as RB contiguous [128,1] columns for the per-rb ptr scalars
    t = nc.declare_dram_parameter("t", [128, RB], F32, isOutput=False)
    t2 = nc.declare_dram_parameter("t2", [RB, 128, 1], F32, isOutput=False)
    out = nc.declare_dram_parameter("out", [1, 1], F32, isOutput=True)

    with tile.TileContext(nc) as tc, ExitStack() as ctx:
        const_pool = ctx.enter_context(tc.tile_pool(name="const", bufs=1))
        xpool = ctx.enter_context(tc.tile_pool(name="x", bufs=2))
        epool = ctx.enter_context(tc.tile_pool(name="e", bufs=2))
        dpool = ctx.enter_context(tc.tile_pool(name="d", bufs=1))
        ppool = ctx.enter_context(tc.tile_pool(name="p", bufs=2))
        jpool = ctx.enter_context(tc.tile_pool(name="j", bufs=1))
        spool = ctx.enter_context(tc.tile_pool(name="s", bufs=1))
        psum_pool = ctx.enter_context(tc.tile_pool(name="ps", bufs=1, space="PSUM"))

        # --- tiny front matter: t block + norm chain on [128, RB] ---------
        ttile = const_pool.tile([128, RB], F32)
        nc.sync.dma_start(out=ttile[:], in_=t[:, :])
        negmu = const_pool.tile([128, 1], F32)
        nc.vector.memset(negmu[:], -MU)
        ones = const_pool.tile([128, 1], F32)
        nc.vector.memset(ones[:], 1.0)

        # fold COEFF into the norm: 1/sqrt(100*n2) = 0.1/sqrt(n2)
        tsq = spool.tile([128, RB], F32)
        nc.scalar.activation(tsq[:], ttile[:], Af.Square, bias=negmu[:])
        n2 = spool.tile([128, RB], F32)
        nc.vector.tensor_scalar(
            n2[:], tsq[:], float(C) / COEFF**2, K / COEFF**2,
            op0=Op.mult, op1=Op.add,
        )
        norm = spool.tile([128, RB], F32)
        nc.scalar.activation(norm[:], n2[:], Af.Sqrt)
        inv = spool.tile([128, RB], F32)
        nc.vector.reciprocal(inv[:], norm[:])
        nz = spool.tile([128, RB], F32)
        nc.vector.tensor_scalar(nz[:], ttile[:], 0.0, None, op0=Op.not_equal)
        scale = spool.tile([128, RB], F32)
        nc.vector.tensor_tensor(scale[:], inv[:], nz[:], op=Op.mult)
        # contiguous [128,1] per-row-block t scalars — a strided column
        # slice of ttile as the ptr-scalar operand blocks the DVE 4x mode
        tcols = []
        for rb in range(RB):
            tc_rb = const_pool.tile([128, 1], F32, tag=f"tc{rb}")
            nc.sync.dma_start(out=tc_rb[:], in_=t2[rb])
            tcols.append(tc_rb)

        # --- constants: one bf16 iota tile per column block, generated in
        # 2048-wide chunks so the first compute tile unblocks early ---------
        iotas = []
        CH = 2048
        for cw in range(NW):
            it = const_pool.tile([128, W], BF16, tag=f"iota{cw}")
            iotas.append(it)
        for ci in range(W // CH):
            for cw in range(NW):
                nc.gpsimd.iota(
                    iotas[cw][:, ci * CH:(ci + 1) * CH],
                    pattern=[[1, CH]],
                    base=cw * W + ci * CH,
                    channel_multiplier=0,
                    allow_small_or_imprecise_dtypes=True,
                )

        # --- main loop over the 8 [128, W] tiles --------------------------
        accM = const_pool.tile([128, NT], F32)
        for idx in range(NT):
            rb, cw = divmod(idx, NW)
            xt = xpool.tile([128, W], F32)
            nc.sync.dma_start(out=xt[:], in_=x[rb, :, cw * W:(cw + 1) * W])
            et = epool.tile([128, W], BF16)
            nc.scalar.activation(et[:], xt[:], Af.Exp)
            dt = dpool.tile([128, W], BF16)
            nc.vector.tensor_scalar(
                dt[:], iotas[cw][:], tcols[rb][:], None, op0=Op.subtract
            )
            pt = ppool.tile([128, W], BF16)
            nc.vector.tensor_tensor(pt[:], dt[:], et[:], op=Op.mult)
            if idx in ACT_IDXS:
                jt = jpool.tile([128, W], BF16)
                nc.scalar.activation(
                    jt[:], pt[:], Af.Abs, accum_out=accM[:, idx:idx + 1]
                )
            else:
                nc.vector.tensor_reduce(
                    accM[:, idx:idx + 1], pt[:], axis=mybir.AxisListType.X,
                    op=Op.add, apply_absolute_value=True,
                )

        # --- combine: rowacc[128, RB] -> scaled -> cross-partition sum ----
        rowacc = spool.tile([128, RB], F32)
        nc.vector.tensor_reduce(
            rowacc[:], accM[:].rearrange("p (rb nw) -> p rb nw", nw=NW),
            axis=mybir.AxisListType.X, op=Op.add,
        )
        partials = spool.tile([128, RB], F32)
        nc.vector.tensor_tensor(partials[:], rowacc[:], scale[:], op=Op.mult)
        ptot = psum_pool.tile([1, RB], F32)
        nc.tensor.matmul(ptot[:], ones[:], partials[:], start=True, stop=True)
        tot = spool.tile([1, 1], F32)
        nc.vector.tensor_reduce(
            tot[:], ptot[:], axis=mybir.AxisListType.X, op=Op.add
        )
        nc.sync.dma_start(out=out[:, :], in_=tot[:])

    nc.finalize()
    return nc


_NC = None


def _get_nc() -> bass.Bass:
    global _NC
    if _NC is None:
        _NC = _build()
    return _NC


def make_in_maps(input: np.ndarray, target: np.ndarray) -> list[dict]:
    x = np.ascontiguousarray(np.asarray(input, dtype=np.float32)).reshape(
        N_CORES, RB, 128, C
    )
    # [N_CORES, 128, RB] partition-major targets + [N_CORES, RB, 128, 1]
    t2 = np.ascontiguousarray(
        np.asarray(target).astype(np.float32).reshape(N_CORES, RB, 128, 1)
    )
    t = np.ascontiguousarray(t2[..., 0].transpose(0, 2, 1))
    return [{"x": x[i], "t": t[i], "t2": t2[i]} for i in range(N_CORES)]


def run(input: np.ndarray, target: np.ndarray, trace: bool = False, tmpdir=None):
    nc = _get_nc()
    in_maps = make_in_maps(input, target)
    res = run_bass_kernel_spmd(
        nc, in_maps, list(range(N_CORES)), trace=trace, tmpdir=tmpdir
    )
    total = np.float32(0.0)
    for r in res.results:
        total += np.float32(r["out"].reshape(-1)[0])
    return np.asarray(total, dtype=np.float32), res


def kernel(input: np.ndarray, target: np.ndarray) -> np.ndarray:
    out, _ = run(input, target)
    return out


# revision 32
# speedup vs baseline: 1.0706x; 1.0706x over previous
"""DistanceFromAnswerLoss on 8 Trainium2 NeuronCores.

out = 0.1 * sum_{b,c} mask[b,c] * exp(input[b,c])
  mask[b,c] = |c - t_b| / sqrt(sum_c (c - t_b)^2),  mask = 0 where t_b == 0

Sharding: data-parallel over the batch dim (512 rows per core); each core
computes its partial 0.1 * sum, host adds the 8 scalars.

Per-core pipeline (memory-bound target: ~47us DMA floor at 360 GB/s):
  ScalarE : e = exp(x) -> bf16 (one pass, 1.2 GHz, table set exp_and_others)
  VectorE : d = iota - t        (tensor_scalar ptr, 2x/4x packed bf16 mode)
            p = d * e           (tensor_tensor, 2x bf16 mode)
  abs+row-reduce, split to balance engines:
    K_ACT tiles: ScalarE activation(Abs, accum_out) — same table set as exp,
                 so no table reloads; gives sum|p| per partition for free.
    rest:        two scalar_tensor_tensor ops (1x) using
                 |c-t|*e = max(c,t)*e - min(c,t)*e.
The row norm uses the closed form sum_c (c-t)^2 = C*(t-mu)^2 + K computed
once for all 512 rows on [128,4] tiles.  Final 128-partition reduction is a
tiny matmul against a ones vector on the otherwise-idle PE.
"""

import sys
from contextlib import ExitStack

import numpy as np

sys.path.insert(0, "/opt/trn_rl_repo")

import concourse.bass as bass
import concourse.tile as tile
from concourse import bacc, mybir
from concourse.bass_utils import run_bass_kernel_spmd

B = 4096
C = 8192
N_CORES = 8
ROWS = B // N_CORES          # 512 rows per core
RB = ROWS // 128             # 4 row blocks of 128 partitions
W = 4096                     # column tile width (2 MiB DMAs)
NW = C // W
NT = RB * NW                 # 8 big tiles per core
ACT_IDXS = {0, 2, 4, 6}      # tiles whose abs+reduce runs on ScalarE
# others: tensor_reduce(apply_absolute_value) on VectorE
COEFF = 0.1

MU = (C - 1) / 2.0
_S1 = (C - 1) * C // 2
_S2 = (C - 1) * C * (2 * C - 1) // 6
K = float(_S2 - _S1 * _S1 / C)   # sum_c (c-t)^2 = C*(t-MU)^2 + K

F32 = mybir.dt.float32
BF16 = mybir.dt.bfloat16
Af = mybir.ActivationFunctionType
Op = mybir.AluOpType


def _build() -> bass.Bass:
    nc = bacc.Bacc("TRN2", target_bir_lowering=False, debug=False)
    x = nc.declare_dram_parameter("x", [RB, 128, C], F32, isOutput=False)
    # t arrives twice: partition-major [128, RB] for the batched norm chain,
    # and as RB contiguous [128,1] columns for the per-rb ptr scalars
    t = nc.declare_dram_parameter("t", [128, RB], F32, isOutput=False)
    t2 = nc.declare_dram_parameter("t2", [RB, 128, 1], F32, isOutput=False)
    out = nc.declare_dram_parameter("out", [1, 1], F32, isOutput=True)

    with tile.TileContext(nc) as tc, ExitStack() as ctx:
        const_pool = ctx.enter_context(tc.tile_pool(name="const", bufs=1))
        xpool = ctx.enter_context(tc.tile_pool(name="x", bufs=4))
        epool = ctx.enter_context(tc.tile_pool(name="e", bufs=3))
        dpool = ctx.enter_context(tc.tile_pool(name="d", bufs=2))
        ppool = ctx.enter_context(tc.tile_pool(name="p", bufs=2))
        jpool = ctx.enter_context(tc.tile_pool(name="j", bufs=2))
        spool = ctx.enter_context(tc.tile_pool(name="s", bufs=1))
        psum_pool = ctx.enter_context(tc.tile_pool(name="ps", bufs=1, space="PSUM"))

        # --- tiny front matter: t block + norm chain on [128, RB] ---------
        ttile = const_pool.tile([128, RB], F32)
        nc.sync.dma_start(out=ttile[:], in_=t[:, :])
        negmu = const_pool.tile([128, 1], F32)
        nc.vector.memset(negmu[:], -MU)
        ones = const_pool.tile([128, 1], F32)
        nc.vector.memset(ones[:], 1.0)

        # fold COEFF into the norm: 1/sqrt(100*n2) = 0.1/sqrt(n2)
        tsq = spool.tile([128, RB], F32)
        nc.scalar.activation(tsq[:], ttile[:], Af.Square, bias=negmu[:])
        n2 = spool.tile([128, RB], F32)
        nc.vector.tensor_scalar(
            n2[:], tsq[:], float(C) / COEFF**2, K / COEFF**2,
            op0=Op.mult, op1=Op.add,
        )
        norm = spool.tile([128, RB], F32)
        nc.scalar.activation(norm[:], n2[:], Af.Sqrt)
        inv = spool.tile([128, RB], F32)
        nc.vector.reciprocal(inv[:], norm[:])
        nz = spool.tile([128, RB], F32)
        nc.vector.tensor_scalar(nz[:], ttile[:], 0.0, None, op0=Op.not_equal)
        scale = spool.tile([128, RB], F32)
        nc.vector.tensor_tensor(scale[:], inv[:], nz[:], op=Op.mult)
        # contiguous [128,1] per-row-block t scalars — a strided column
        # slice of ttile as the ptr-scalar operand blocks the DVE 4x mode
        tcols = []
        for rb in range(RB):
            tc_rb = const_pool.tile([128, 1], F32, tag=f"tc{rb}")
            nc.sync.dma_start(out=tc_rb[:], in_=t2[rb])
            tcols.append(tc_rb)

        # --- constants: bf16 iota generated in 2048-wide chunks so the
        # first compute tile unblocks early --------------------------------
        iota = const_pool.tile([128, C], BF16)
        CH = 2048
        for ci in range(C // CH):
            nc.gpsimd.iota(
                iota[:, ci * CH:(ci + 1) * CH],
                pattern=[[1, CH]],
                base=ci * CH,
                channel_multiplier=0,
                allow_small_or_imprecise_dtypes=True,
            )

        # --- main loop: dist per row-block, then the 8 [128, W] tiles -----
        # separate per-engine accumulator tiles: a single shared acc tile
        # written by both ACT and DVE breeds conservative cross-engine deps
        accA = const_pool.tile([128, NT], F32)
        accD = const_pool.tile([128, NT], F32)
        nc.vector.memset(accA[:], 0.0)
        nc.vector.memset(accD[:], 0.0)
        dts = []
        for rb in range(RB):
            dtf = dpool.tile([128, C], BF16)
            nc.vector.tensor_scalar(
                dtf[:], iota[:], tcols[rb][:], None, op0=Op.subtract
            )
            dts.append(dtf)
        for idx in range(NT):
            rb, cw = divmod(idx, NW)
            xt = xpool.tile([128, W], F32)
            nc.sync.dma_start(out=xt[:], in_=x[rb, :, cw * W:(cw + 1) * W])
            et = epool.tile([128, W], BF16)
            nc.scalar.activation(et[:], xt[:], Af.Exp)
            pt = ppool.tile([128, W], BF16)
            nc.vector.tensor_tensor(
                pt[:], dts[rb][:, cw * W:(cw + 1) * W], et[:], op=Op.mult
            )
            if idx in ACT_IDXS:
                jt = jpool.tile([128, W], BF16)
                nc.scalar.activation(
                    jt[:], pt[:], Af.Abs, accum_out=accA[:, idx:idx + 1]
                )
            else:
                nc.vector.tensor_reduce(
                    accD[:, idx:idx + 1], pt[:], axis=mybir.AxisListType.X,
                    op=Op.add, apply_absolute_value=True,
                )

        # --- combine: rowacc[128, RB] -> scaled -> cross-partition sum ----
        # ACT-path tiles are the even idx, DVE-path the odd: per row block
        # rb, cols {rb*NW..} of accA/accD hold its partials (unused cols of
        # each tile are never read)
        accsum = spool.tile([128, NT], F32)
        nc.vector.tensor_add(accsum[:], accA[:], accD[:])
        rowacc = spool.tile([128, RB], F32)
        nc.vector.tensor_reduce(
            rowacc[:], accsum[:].rearrange("p (rb nw) -> p rb nw", nw=NW),
            axis=mybir.AxisListType.X, op=Op.add,
        )
        partials = spool.tile([128, RB], F32)
        nc.vector.tensor_tensor(partials[:], rowacc[:], scale[:], op=Op.mult)
        ptot = psum_pool.tile([1, RB], F32)
        nc.tensor.matmul(ptot[:], ones[:], partials[:], start=True, stop=True)
        tot = spool.tile([1, 1], F32)
        nc.vector.tensor_reduce(
            tot[:], ptot[:], axis=mybir.AxisListType.X, op=Op.add
        )
        nc.sync.dma_start(out=out[:, :], in_=tot[:])

    nc.finalize()
    return nc


_NC = None


def _get_nc() -> bass.Bass:
    global _NC
    if _NC is None:
        _NC = _build()
    return _NC


def make_in_maps(input: np.ndarray, target: np.ndarray) -> list[dict]:
    x = np.ascontiguousarray(np.asarray(input, dtype=np.float32)).reshape(
        N_CORES, RB, 128, C
    )
    # [N_CORES, 128, RB] partition-major targets + [N_CORES, RB, 128, 1]
    t2 = np.ascontiguousarray(
        np.asarray(target).astype(np.float32).reshape(N_CORES, RB, 128, 1)
    )
    t = np.ascontiguousarray(t2[..., 0].transpose(0, 2, 1))
    return [{"x": x[i], "t": t[i], "t2": t2[i]} for i in range(N_CORES)]


def run(input: np.ndarray, target: np.ndarray, trace: bool = False, tmpdir=None):
    nc = _get_nc()
    in_maps = make_in_maps(input, target)
    res = run_bass_kernel_spmd(
        nc, in_maps, list(range(N_CORES)), trace=trace, tmpdir=tmpdir
    )
    total = np.float32(0.0)
    for r in res.results:
        total += np.float32(r["out"].reshape(-1)[0])
    return np.asarray(total, dtype=np.float32), res


def kernel(input: np.ndarray, target: np.ndarray) -> np.ndarray:
    out, _ = run(input, target)
    return out
